# revision 1
# baseline (speedup 1.0000x reference)
import numpy as np
from numpy.lib.stride_tricks import sliding_window_view

# nn_BatchFFTMA: H*W = 9216 independent 65x65 FFT-MA simulations.
# Shapes hardcoded per spec: angle_matrix [96,96] f32, noise [1,1,160,160] f32.
H, W = 96, 96
D = 32
WIN = 2 * D + 1  # 65
A_, B_ = 15.0, 3.0
EXP = 0.5
N_SHARDS = 8  # window/batch axis split matching the 8-core data-parallel plan


def _simulate_chunk(patches, theta, Xm, Ym):
    # patches: [n, 65, 65], theta: [n]
    c = np.cos(theta)[:, None, None]
    s = np.sin(theta)[:, None, None]
    a_part = (Xm[None] * c + Ym[None] * s) ** 2 / (A_**2)
    b_part = (-Xm[None] * s + Ym[None] * c) ** 2 / (B_**2)
    R = np.exp(-((a_part + b_part) ** EXP))
    Wf = np.fft.fft2(patches)
    Rf = np.fft.fft2(np.fft.fftshift(R, axes=(-2, -1)))
    G = np.sqrt(Rf + 1e-8)
    v = np.fft.ifft2(Wf * G).real
    mean = v.mean(axis=(1, 2), keepdims=True)
    std = v.std(axis=(1, 2), keepdims=True, ddof=1)
    v = (v - mean) / (std + 1e-6)
    return v[:, D, D]


def kernel(angle_matrix, noise):
    angle = np.asarray(angle_matrix, dtype=np.float32)
    noise2d = np.asarray(noise, dtype=np.float32)[0, 0]
    patches = sliding_window_view(noise2d, (WIN, WIN)).reshape(H * W, WIN, WIN)
    theta = angle.reshape(-1).astype(np.float64)
    x = np.linspace(-D, D, WIN)
    Xm, Ym = np.meshgrid(x, x, indexing="ij")

    L = H * W
    out = np.empty(L, dtype=np.float32)
    step = L // N_SHARDS
    for i in range(0, L, step):
        out[i : i + step] = _simulate_chunk(
            patches[i : i + step].astype(np.float64), theta[i : i + step], Xm, Ym
        )
    return out.reshape(H, W)



# revision 12
# speedup vs baseline: 6.0234x; 6.0234x over previous
"""Trainium2 Bass kernel for nn_BatchFFTMA: 9216 independent 65x65 FFT-MA sims.

Math (validated in proto.py against the jax reference):
  For each window w (patch p = noise[r0:r0+65, c0:c0+65], angle theta):
    Wf' = Cpt^T p Cpt   with Cpt = F*diag((-1)^k)  (patch DFT; center-pixel
          phase e^{2pi i 32k/65} and the principal-sqrt half-shift phase
          combine to exactly (-1)^k)
    E   = Re(Cq^T R Cq) with Cq = F*diag(e^{-2pi i 33 k/65})  (DFT of the
          ifftshift-aligned R -> real, even spectrum; the reference's
          fftshift-vs-ifftshift off-by-one is the source of the half-shift)
    R   = exp(-sqrt(q)), q = alpha*x_r^2 + beta*x_c^2 + gamma*x_r*x_c
    gp  = sqrt(relu(E+1e-8)), gn = sqrt(relu(-(E+1e-8)))
    g+  = gp*SGP, g- = gn*SGN   (SGP/SGN: +-1 fields from sqrt branch cuts)
    a = Wf'_r*g+, b = Wf'_i*g-; Xr = a - b
    VC = sum(Xr); X00 = Xr[0,0]; S = sum((Wf'_r^2+Wf'_i^2)*|E+1e-8|)
    out_w = ((VC-X00)/N^2) / (sqrt((S-X00^2)/(N^2(N^2-1))) + 1e-6)
  (v = ifft2 never materialized: center pixel via phase fold, mean via X[0,0],
   std via Parseval.)

Sharding: window/batch axis across 8 cores (1152 windows each, 12 output rows).
Precision: patch DFT + spectral chain bf16 (validated), R field + R DFT fp32.
"""
import os
import numpy as np
import ml_dtypes

H, W, D = 96, 96, 32
N = 65
N2 = N * N
A_, B_ = 15.0, 3.0
NCORE = 8
WPC = H * W // NCORE      # 1152 windows per core
RPC = H // NCORE          # 12 output rows per core
CB = 18                   # windows per vector chunk
GRP = 3                   # windows per matmul/PSUM group
NGRP = CB // GRP

_bf16 = ml_dtypes.bfloat16


def _host_constants():
    k = np.arange(N)
    F = np.exp(-2j * np.pi * np.outer(k, k) / N)
    Cpt = F * ((-1.0) ** k)[None, :]
    Cq = F * np.exp(-2j * np.pi * k * 33 / N)[None, :]
    Cr = Cpt.real.astype(np.float32)
    Ci = Cpt.imag.astype(np.float32)
    Qr = Cq.real.astype(np.float32)
    Qi = Cq.imag.astype(np.float32)
    pconst = np.concatenate([Cr, Ci, -Ci, Cr], axis=1).astype(_bf16)   # [65, 260]
    rconst = np.concatenate([Qr, Qi, -Qi], axis=1).astype(np.float32)  # [65, 195]

    k1, k2 = np.meshgrid(k, k, indexing="ij")
    ksum = k1 + k2
    m = ksum % N
    extra = (-1.0) ** (ksum // N)
    SGP = extra * np.where(m <= 32, 1.0, -1.0)
    SGN = extra * np.where(m == 0, 1.0, -1.0)
    sgp_t = np.tile(SGP, (1, CB)).astype(_bf16)   # [65, 65*CB]
    sgn_t = np.tile(SGN, (1, CB)).astype(_bf16)
    sgconst = np.concatenate([sgp_t, sgn_t], axis=1)  # [65, 2*65*CB]
    return pconst, rconst, sgconst


def _build_program(nchunk):
    import concourse.bacc as bacc
    import concourse.mybir as mybir
    from concourse.tile import TileContext

    f32 = mybir.dt.float32
    bf16 = mybir.dt.bfloat16
    AF = mybir.ActivationFunctionType
    ALU = mybir.AluOpType
    AX = mybir.AxisListType

    nwin = nchunk * CB
    nc = bacc.Bacc()
    strips_in = nc.declare_dram_parameter("strips", [N, 12 * 160], bf16, isOutput=False)
    q_in = nc.declare_dram_parameter("qfield", [N, nwin * N], f32, isOutput=False)
    pconst_in = nc.declare_dram_parameter("pconst", [N, 260], bf16, isOutput=False)
    rconst_in = nc.declare_dram_parameter("rconst", [N, 195], f32, isOutput=False)
    sg_in = nc.declare_dram_parameter("sgconst", [N, 2 * N * CB], bf16, isOutput=False)
    out_d = nc.declare_dram_parameter("out", [131, nwin], f32, isOutput=True)

    with TileContext(nc) as tc:
        with (
            tc.tile_pool(name="const", bufs=1) as cpool,
            tc.tile_pool(name="qsb", bufs=8) as qpool,
            tc.tile_pool(name="rsb", bufs=2) as rpool,
            tc.tile_pool(name="o1p", bufs=2) as o1ppool,
            tc.tile_pool(name="o1r", bufs=2) as o1rpool,
            tc.tile_pool(name="wfsb", bufs=2) as wfpool,
            tc.tile_pool(name="esb", bufs=2) as epool,
            tc.tile_pool(name="spec", bufs=2) as spool,
            tc.tile_pool(name="ps1p", bufs=2, space="PSUM") as pp1,
            tc.tile_pool(name="ps1r", bufs=2, space="PSUM") as pr1,
            tc.tile_pool(name="pswf", bufs=2, space="PSUM") as pwf,
            tc.tile_pool(name="pse", bufs=2, space="PSUM") as pe_,
        ):
            pc = cpool.tile([N, 260], bf16)
            nc.sync.dma_start(out=pc[:], in_=pconst_in[:])
            rc = cpool.tile([N, 195], f32)
            nc.sync.dma_start(out=rc[:], in_=rconst_in[:])
            sg = cpool.tile([N, 2 * N * CB], bf16)
            nc.sync.dma_start(out=sg[:], in_=sg_in[:])

            strips = cpool.tile([N, 12 * 160], bf16)
            nc.sync.dma_start(out=strips[:], in_=strips_in[:])

            # clock warm-up: make ACT/DVE observe every setup DMA queue via
            # tiny reads, so later real instructions emit few sync waits
            # (walrus caps waits per instruction; vector clocks here are not
            # transitively collapsed).
            warm_s = cpool.tile([1, 4], f32, tag="warm_s")
            warm_v = cpool.tile([1, 4], f32, tag="warm_v")
            for idx, src in enumerate((strips, pc, rc, sg)):
                nc.scalar.copy(warm_s[0:1, idx:idx + 1], src[0:1, 0:1])
                nc.vector.tensor_copy(warm_v[0:1, idx:idx + 1], src[0:1, 0:1])

            vcs = cpool.tile([N, nwin], f32)
            ss = cpool.tile([N, nwin], f32)
            mus = cpool.tile([1, nwin], f32)

            for ch in range(nchunk):
                cw0 = ch * CB
                q_sb = qpool.tile([N, CB * N], f32)
                nc.gpsimd.dma_start(out=q_sb[:], in_=q_in[:, cw0 * N:(cw0 + CB) * N])
                s_sb = qpool.tile([N, CB * N], f32, tag="s_sb")
                nc.scalar.activation(s_sb[:], q_sb[:], AF.Sqrt)
                r_sb = rpool.tile([N, CB * N], f32)
                nc.scalar.activation(r_sb[:], s_sb[:], AF.Exp, scale=-1.0)

                wf_sb = wfpool.tile([N, CB * 130], bf16)
                e_sb = epool.tile([N, CB * N], bf16)

                for g in range(NGRP):
                    ps1p = pp1.tile([N, 390], f32)
                    ps1r = pr1.tile([N, 390], f32)
                    wfp = pwf.tile([N, 390], f32)
                    ep = pe_.tile([N, 195], f32)
                    for j in range(GRP):
                        w = cw0 + g * GRP + j
                        r0, c0 = divmod(w, 96)
                        lhs = strips[:, r0 * 160 + c0: r0 * 160 + c0 + N]
                        nc.tensor.matmul(
                            ps1p[:, j * 130:(j + 1) * 130], lhs, pc[:, 0:130],
                            start=True, stop=True,
                        )
                    o1p = o1ppool.tile([N, 390], bf16)
                    nc.scalar.copy(o1p[:], ps1p[:])
                    for j in range(GRP):
                        nc.tensor.matmul(
                            wfp[:, j * 130:(j + 1) * 130],
                            o1p[:, j * 130:j * 130 + N], pc[:, 0:130],
                            start=True, stop=False,
                        )
                        nc.tensor.matmul(
                            wfp[:, j * 130:(j + 1) * 130],
                            o1p[:, j * 130 + N:(j + 1) * 130], pc[:, 130:260],
                            start=False, stop=True,
                        )
                    nc.scalar.copy(wf_sb[:, g * 390:(g + 1) * 390], wfp[:])

                    for j in range(GRP):
                        wl = (g * GRP + j) * N
                        nc.tensor.matmul(
                            ps1r[:, j * 130:(j + 1) * 130],
                            r_sb[:, wl:wl + N], rc[:, 0:130],
                            start=True, stop=True,
                        )
                    o1r = o1rpool.tile([N, 390], f32)
                    nc.vector.tensor_copy(o1r[:], ps1r[:])
                    for j in range(GRP):
                        nc.tensor.matmul(
                            ep[:, j * N:(j + 1) * N],
                            o1r[:, j * 130:j * 130 + N], rc[:, 0:N],
                            start=True, stop=False,
                        )
                        nc.tensor.matmul(
                            ep[:, j * N:(j + 1) * N],
                            o1r[:, j * 130 + N:(j + 1) * 130], rc[:, 130:195],
                            start=False, stop=True,
                        )
                    nc.vector.tensor_copy(e_sb[:, g * 195:(g + 1) * 195], ep[:])

                # spectral chain, batched over the CB windows of this chunk
                FD = CB * N
                rp = spool.tile([N, FD], bf16, tag="rp")
                nc.vector.tensor_scalar(rp[:], e_sb[:], 1e-8, 0.0, op0=ALU.add, op1=ALU.max)
                mn = spool.tile([N, FD], bf16, tag="mn")
                nc.vector.tensor_scalar(mn[:], e_sb[:], 1e-8, 0.0, op0=ALU.add, op1=ALU.min)
                gp = spool.tile([N, FD], bf16, tag="gp")
                nc.scalar.activation(gp[:], rp[:], AF.Sqrt)
                gn = spool.tile([N, FD], bf16, tag="gn")
                nc.scalar.activation(gn[:], mn[:], AF.Sqrt, scale=-1.0)
                gps = spool.tile([N, FD], bf16, tag="gps")
                nc.vector.tensor_mul(gps[:], gp[:], sg[:, 0:FD])
                gns = spool.tile([N, FD], bf16, tag="gns")
                nc.vector.tensor_mul(gns[:], gn[:], sg[:, FD:2 * FD])

                wf3 = wf_sb[:].rearrange("p (w t c) -> p w t c", t=2, c=N)
                wr = wf3[:, :, 0, :]   # [65, CB, 65]
                wi = wf3[:, :, 1, :]
                gps3 = gps[:].rearrange("p (w c) -> p w c", c=N)
                gns3 = gns[:].rearrange("p (w c) -> p w c", c=N)
                a_t = spool.tile([N, FD], bf16, tag="a_t")
                a3 = a_t[:].rearrange("p (w c) -> p w c", c=N)
                nc.vector.tensor_mul(a3, wr, gps3)
                b_t = spool.tile([N, FD], bf16, tag="b_t")
                b3 = b_t[:].rearrange("p (w c) -> p w c", c=N)
                nc.vector.tensor_mul(b3, wi, gns3)
                xr = spool.tile([N, FD], bf16, tag="xr")
                nc.vector.tensor_sub(xr[:], a_t[:], b_t[:])

                wr2 = spool.tile([N, FD], bf16, tag="wr2")
                wr23 = wr2[:].rearrange("p (w c) -> p w c", c=N)
                nc.vector.tensor_mul(wr23, wr, wr)
                wi2 = spool.tile([N, FD], bf16, tag="wi2")
                wi23 = wi2[:].rearrange("p (w c) -> p w c", c=N)
                nc.vector.tensor_mul(wi23, wi, wi)
                w2s = spool.tile([N, FD], bf16, tag="w2s")
                nc.vector.tensor_add(w2s[:], wr2[:], wi2[:])
                az = spool.tile([N, FD], bf16, tag="az")
                nc.vector.tensor_sub(az[:], rp[:], mn[:])   # |E+eps| = relu(z) - min(z,0)
                st = spool.tile([N, FD], bf16, tag="st")
                nc.vector.tensor_mul(st[:], w2s[:], az[:])

                xr3 = xr[:].rearrange("p (w c) -> p w c", c=N)
                st3 = st[:].rearrange("p (w c) -> p w c", c=N)
                nc.vector.tensor_reduce(
                    vcs[:, cw0:cw0 + CB], xr3, axis=AX.X, op=ALU.add
                )
                nc.vector.tensor_reduce(
                    ss[:, cw0:cw0 + CB], st3, axis=AX.X, op=ALU.add
                )
                nc.vector.tensor_copy(mus[0:1, cw0:cw0 + CB], xr3[0:1, :, 0])

            nc.sync.dma_start(out=out_d[0:N, :], in_=vcs[:])
            nc.sync.dma_start(out=out_d[N:130, :], in_=ss[:])
            nc.sync.dma_start(out=out_d[130:131, :], in_=mus[:])
    if not nc.is_finalized():
        nc.finalize()
    return nc


def _host_inputs(angle_matrix, noise, nchunk):
    """Per-core input maps. Core c owns output rows [12c, 12c+12)."""
    pconst, rconst, sgconst = _host_constants()
    noise2d = np.ascontiguousarray(np.asarray(noise, dtype=np.float32)[0, 0])
    ang = np.asarray(angle_matrix, dtype=np.float32).reshape(-1).astype(np.float64)
    c = np.cos(ang); s = np.sin(ang)
    alpha = (c * c / A_**2 + s * s / B_**2).astype(np.float32)
    beta = (s * s / A_**2 + c * c / B_**2).astype(np.float32)
    gamma = (2 * c * s * (1 / A_**2 - 1 / B_**2)).astype(np.float32)
    x = np.linspace(-D, D, N, dtype=np.float32)
    x2 = x * x
    nwin = nchunk * CB

    in_maps = []
    for core in range(NCORE):
        w0 = core * WPC
        al = alpha[w0:w0 + nwin]
        be = beta[w0:w0 + nwin]
        ga = gamma[w0:w0 + nwin]
        # q[r, w, c] = x2[r]*al[w] + x2[c]*be[w] + x[r]*x[c]*ga[w]
        q = (
            x2[:, None, None] * al[None, :, None]
            + x2[None, None, :] * be[None, :, None]
            + (x[:, None, None] * x[None, None, :]) * ga[None, :, None]
        )
        np.maximum(q, 0.0, out=q)
        r_base = core * RPC
        strips = np.concatenate(
            [noise2d[r_base + i: r_base + i + N, :] for i in range(12)], axis=1
        ).astype(_bf16)
        in_maps.append({
            "strips": strips,
            "qfield": np.ascontiguousarray(q.reshape(N, -1)),
            "pconst": pconst,
            "rconst": rconst,
            "sgconst": sgconst,
        })
    return in_maps


def _finalize(core_outs):
    """core_outs: list of [131, nwin] arrays -> [96, 96] output."""
    blocks = []
    for arr in core_outs:
        vc = arr[0:N].sum(axis=0)
        s_ = arr[N:130].sum(axis=0)
        mu = arr[130]
        vcn = (vc - mu) / N2
        var = (s_ - mu * mu) / (N2 * (N2 - 1.0))
        outrow = vcn / (np.sqrt(np.maximum(var, 0.0)) + 1e-6)
        blocks.append(outrow.reshape(-1, 96))
    return np.concatenate(blocks, axis=0).astype(np.float32)


_PROG = {}


def _get_program(nchunk):
    if nchunk not in _PROG:
        _PROG[nchunk] = _build_program(nchunk)
    return _PROG[nchunk]


def kernel(angle_matrix, noise):
    from concourse.bass_utils import run_bass_kernel_spmd

    nchunk = WPC // CB
    nc = _get_program(nchunk)
    in_maps = _host_inputs(angle_matrix, noise, nchunk)
    res = run_bass_kernel_spmd(nc, in_maps, core_ids=list(range(NCORE)))
    core_outs = [res.results[i]["out"] for i in range(NCORE)]
    return _finalize(core_outs)


# revision 17
# speedup vs baseline: 14.5606x; 2.4174x over previous
"""Trainium2 Bass kernel for nn_BatchFFTMA: 9216 independent 65x65 FFT-MA sims.

Math (validated in proto.py against the jax reference):
  For each window w (patch p = noise[r0:r0+65, c0:c0+65], angle theta):
    Wf' = Cpt^T p Cpt   with Cpt = F*diag((-1)^k)  (patch DFT; center-pixel
          phase e^{2pi i 32k/65} and the principal-sqrt half-shift phase
          combine to exactly (-1)^k)
    E   = Re(Cq^T R Cq) with Cq = F*diag(e^{-2pi i 33 k/65})  (DFT of the
          ifftshift-aligned R -> real, even spectrum; the reference's
          fftshift-vs-ifftshift off-by-one is the source of the half-shift)
    R   = exp(-sqrt(q)), q = alpha*x_r^2 + beta*x_c^2 + gamma*x_r*x_c
    gp  = sqrt(relu(E+1e-8)), gn = sqrt(relu(-(E+1e-8)))
    g+  = gp*SGP, g- = gn*SGN   (SGP/SGN: +-1 fields from sqrt branch cuts)
    a = Wf'_r*g+, b = Wf'_i*g-; Xr = a - b
    VC = sum(Xr); X00 = Xr[0,0]; S = sum((Wf'_r^2+Wf'_i^2)*|E+1e-8|)
    out_w = ((VC-X00)/N^2) / (sqrt((S-X00^2)/(N^2(N^2-1))) + 1e-6)
  (v = ifft2 never materialized: center pixel via phase fold, mean via X[0,0],
   std via Parseval.)

Sharding: window/batch axis across 8 cores (1152 windows each, 12 output rows).
Precision: patch DFT + spectral chain bf16 (validated), R field + R DFT fp32.
"""
import os
import numpy as np
import ml_dtypes

H, W, D = 96, 96, 32
N = 65
N2 = N * N
A_, B_ = 15.0, 3.0
NCORE = 8
WPC = H * W // NCORE      # 1152 windows per core
RPC = H // NCORE          # 12 output rows per core
CB = 18                   # windows per vector chunk
GRP = 3                   # windows per matmul/PSUM group
NGRP = CB // GRP

_bf16 = ml_dtypes.bfloat16


def _host_constants():
    k = np.arange(N)
    F = np.exp(-2j * np.pi * np.outer(k, k) / N)
    Cpt = F * ((-1.0) ** k)[None, :]
    Cq = F * np.exp(-2j * np.pi * k * 33 / N)[None, :]
    Cr = Cpt.real.astype(np.float32)
    Ci = Cpt.imag.astype(np.float32)
    Qr = Cq.real.astype(np.float32)
    Qi = Cq.imag.astype(np.float32)
    pconst = np.concatenate([Cr, Ci, -Ci, Cr], axis=1).astype(_bf16)   # [65, 260]
    rconst = np.concatenate([Qr, Qi, -Qi], axis=1).astype(np.float32)  # [65, 195]

    k1, k2 = np.meshgrid(k, k, indexing="ij")
    ksum = k1 + k2
    m = ksum % N
    extra = (-1.0) ** (ksum // N)
    SGP = extra * np.where(m <= 32, 1.0, -1.0)
    SGN = extra * np.where(m == 0, 1.0, -1.0)
    sgp_t = np.tile(SGP, (1, CB)).astype(_bf16)   # [65, 65*CB]
    sgn_t = np.tile(SGN, (1, CB)).astype(_bf16)
    sgconst = np.concatenate([sgp_t, sgn_t], axis=1)  # [65, 2*65*CB]
    return pconst, rconst, sgconst


def _build_program(nchunk):
    import concourse.bacc as bacc
    import concourse.mybir as mybir
    from concourse.tile import TileContext

    f32 = mybir.dt.float32
    bf16 = mybir.dt.bfloat16
    AF = mybir.ActivationFunctionType
    ALU = mybir.AluOpType
    AX = mybir.AxisListType

    nwin = nchunk * CB
    nc = bacc.Bacc()
    strips_in = nc.declare_dram_parameter("strips", [N, 12 * 160], bf16, isOutput=False)
    rhsq_in = nc.declare_dram_parameter("rhsq", [3, nwin * N], f32, isOutput=False)
    qbasis_in = nc.declare_dram_parameter("qbasis", [3, N], f32, isOutput=False)
    pconst_in = nc.declare_dram_parameter("pconst", [N, 260], bf16, isOutput=False)
    rconst_in = nc.declare_dram_parameter("rconst", [N, 195], f32, isOutput=False)
    sg_in = nc.declare_dram_parameter("sgconst", [N, 2 * N * CB], bf16, isOutput=False)
    out_d = nc.declare_dram_parameter("out", [131, nwin], f32, isOutput=True)

    with TileContext(nc) as tc:
        with (
            tc.tile_pool(name="const", bufs=1) as cpool,
            tc.tile_pool(name="qsb", bufs=8) as qpool,
            tc.tile_pool(name="rsb", bufs=2) as rpool,
            tc.tile_pool(name="o1p", bufs=2) as o1ppool,
            tc.tile_pool(name="o1r", bufs=2) as o1rpool,
            tc.tile_pool(name="wfsb", bufs=2) as wfpool,
            tc.tile_pool(name="esb", bufs=2) as epool,
            tc.tile_pool(name="spec", bufs=2) as spool,
            tc.tile_pool(name="psq", bufs=2, space="PSUM") as pq_,
            tc.tile_pool(name="ps1p", bufs=2, space="PSUM") as pp1,
            tc.tile_pool(name="ps1r", bufs=2, space="PSUM") as pr1,
            tc.tile_pool(name="pswf", bufs=1, space="PSUM") as pwf,
            tc.tile_pool(name="pse", bufs=1, space="PSUM") as pe_,
        ):
            pc = cpool.tile([N, 260], bf16)
            nc.sync.dma_start(out=pc[:], in_=pconst_in[:])
            rc = cpool.tile([N, 195], f32)
            nc.sync.dma_start(out=rc[:], in_=rconst_in[:])
            sg = cpool.tile([N, 2 * N * CB], bf16)
            nc.sync.dma_start(out=sg[:], in_=sg_in[:])

            strips = cpool.tile([N, 12 * 160], bf16)
            nc.sync.dma_start(out=strips[:], in_=strips_in[:])
            qbasis = cpool.tile([3, N], f32)
            nc.sync.dma_start(out=qbasis[:], in_=qbasis_in[:])

            # clock warm-up: make ACT/DVE observe every setup DMA queue via
            # tiny reads, so later real instructions emit few sync waits
            # (walrus caps waits per instruction; vector clocks here are not
            # transitively collapsed).
            warm_s = cpool.tile([1, 4], f32, tag="warm_s")
            warm_v = cpool.tile([1, 4], f32, tag="warm_v")
            for idx, src in enumerate((strips, pc, rc, sg)):
                nc.scalar.copy(warm_s[0:1, idx:idx + 1], src[0:1, 0:1])
                nc.vector.tensor_copy(warm_v[0:1, idx:idx + 1], src[0:1, 0:1])

            vcs = cpool.tile([N, nwin], f32)
            ss = cpool.tile([N, nwin], f32)
            mus = cpool.tile([1, nwin], f32)

            for ch in range(nchunk):
                cw0 = ch * CB
                rq_sb = qpool.tile([3, CB * N], f32)
                nc.gpsimd.dma_start(
                    out=rq_sb[:], in_=rhsq_in[:, cw0 * N:(cw0 + CB) * N]
                )
                s_sb = qpool.tile([N, CB * N], f32, tag="s_sb")
                for h in range(3):
                    qps = pq_.tile([N, 390], f32)
                    nc.tensor.matmul(
                        qps[:], qbasis[:], rq_sb[:, h * 390:(h + 1) * 390],
                        start=True, stop=True,
                    )
                    # sqrt straight out of PSUM into the chunk-wide s tile
                    nc.scalar.activation(
                        s_sb[:, h * 390:(h + 1) * 390], qps[:], AF.Sqrt
                    )
                r_sb = rpool.tile([N, CB * N], f32)
                nc.scalar.activation(r_sb[:], s_sb[:], AF.Exp, scale=-1.0)

                wf_sb = wfpool.tile([N, CB * 130], bf16)
                e_sb = epool.tile([N, CB * N], bf16)

                for g in range(NGRP):
                    ps1p = pp1.tile([N, 390], f32)
                    ps1r = pr1.tile([N, 390], f32)
                    wfp = pwf.tile([N, 390], f32)
                    ep = pe_.tile([N, 195], f32)
                    for j in range(GRP):
                        w = cw0 + g * GRP + j
                        r0, c0 = divmod(w, 96)
                        lhs = strips[:, r0 * 160 + c0: r0 * 160 + c0 + N]
                        nc.tensor.matmul(
                            ps1p[:, j * 130:(j + 1) * 130], lhs, pc[:, 0:130],
                            start=True, stop=True,
                        )
                    o1p = o1ppool.tile([N, 390], bf16)
                    nc.scalar.copy(o1p[:], ps1p[:])
                    for j in range(GRP):
                        nc.tensor.matmul(
                            wfp[:, j * 130:(j + 1) * 130],
                            o1p[:, j * 130:j * 130 + N], pc[:, 0:130],
                            start=True, stop=False,
                        )
                        nc.tensor.matmul(
                            wfp[:, j * 130:(j + 1) * 130],
                            o1p[:, j * 130 + N:(j + 1) * 130], pc[:, 130:260],
                            start=False, stop=True,
                        )
                    nc.scalar.copy(wf_sb[:, g * 390:(g + 1) * 390], wfp[:])

                    for j in range(GRP):
                        wl = (g * GRP + j) * N
                        nc.tensor.matmul(
                            ps1r[:, j * 130:(j + 1) * 130],
                            r_sb[:, wl:wl + N], rc[:, 0:130],
                            start=True, stop=True,
                        )
                    o1r = o1rpool.tile([N, 390], f32)
                    nc.vector.tensor_copy(o1r[:], ps1r[:])
                    for j in range(GRP):
                        nc.tensor.matmul(
                            ep[:, j * N:(j + 1) * N],
                            o1r[:, j * 130:j * 130 + N], rc[:, 0:N],
                            start=True, stop=False,
                        )
                        nc.tensor.matmul(
                            ep[:, j * N:(j + 1) * N],
                            o1r[:, j * 130 + N:(j + 1) * 130], rc[:, 130:195],
                            start=False, stop=True,
                        )
                    nc.vector.tensor_copy(e_sb[:, g * 195:(g + 1) * 195], ep[:])

                # spectral chain, batched over the CB windows of this chunk
                FD = CB * N
                rp = spool.tile([N, FD], bf16, tag="rp")
                nc.vector.tensor_scalar(rp[:], e_sb[:], 1e-8, 0.0, op0=ALU.add, op1=ALU.max)
                mn = spool.tile([N, FD], bf16, tag="mn")
                nc.vector.tensor_scalar(mn[:], e_sb[:], 1e-8, 0.0, op0=ALU.add, op1=ALU.min)
                gp = spool.tile([N, FD], bf16, tag="gp")
                nc.scalar.activation(gp[:], rp[:], AF.Sqrt)
                gn = spool.tile([N, FD], bf16, tag="gn")
                nc.scalar.activation(gn[:], mn[:], AF.Sqrt, scale=-1.0)
                gps = spool.tile([N, FD], bf16, tag="gps")
                nc.vector.tensor_mul(gps[:], gp[:], sg[:, 0:FD])
                gns = spool.tile([N, FD], bf16, tag="gns")
                nc.vector.tensor_mul(gns[:], gn[:], sg[:, FD:2 * FD])

                wf3 = wf_sb[:].rearrange("p (w t c) -> p w t c", t=2, c=N)
                wr = wf3[:, :, 0, :]   # [65, CB, 65]
                wi = wf3[:, :, 1, :]
                gps3 = gps[:].rearrange("p (w c) -> p w c", c=N)
                gns3 = gns[:].rearrange("p (w c) -> p w c", c=N)
                a_t = spool.tile([N, FD], bf16, tag="a_t")
                a3 = a_t[:].rearrange("p (w c) -> p w c", c=N)
                nc.vector.tensor_mul(a3, wr, gps3)
                b_t = spool.tile([N, FD], bf16, tag="b_t")
                b3 = b_t[:].rearrange("p (w c) -> p w c", c=N)
                nc.vector.tensor_mul(b3, wi, gns3)
                xr = spool.tile([N, FD], bf16, tag="xr")
                nc.vector.tensor_sub(xr[:], a_t[:], b_t[:])

                wr2 = spool.tile([N, FD], bf16, tag="wr2")
                wr23 = wr2[:].rearrange("p (w c) -> p w c", c=N)
                nc.vector.tensor_mul(wr23, wr, wr)
                wi2 = spool.tile([N, FD], bf16, tag="wi2")
                wi23 = wi2[:].rearrange("p (w c) -> p w c", c=N)
                nc.vector.tensor_mul(wi23, wi, wi)
                w2s = spool.tile([N, FD], bf16, tag="w2s")
                nc.vector.tensor_add(w2s[:], wr2[:], wi2[:])
                az = spool.tile([N, FD], bf16, tag="az")
                nc.vector.tensor_sub(az[:], rp[:], mn[:])   # |E+eps| = relu(z) - min(z,0)
                st = spool.tile([N, FD], bf16, tag="st")
                nc.vector.tensor_mul(st[:], w2s[:], az[:])

                xr3 = xr[:].rearrange("p (w c) -> p w c", c=N)
                st3 = st[:].rearrange("p (w c) -> p w c", c=N)
                nc.vector.tensor_reduce(
                    vcs[:, cw0:cw0 + CB], xr3, axis=AX.X, op=ALU.add
                )
                nc.vector.tensor_reduce(
                    ss[:, cw0:cw0 + CB], st3, axis=AX.X, op=ALU.add
                )
                nc.vector.tensor_copy(mus[0:1, cw0:cw0 + CB], xr3[0:1, :, 0])

            nc.sync.dma_start(out=out_d[0:N, :], in_=vcs[:])
            nc.sync.dma_start(out=out_d[N:130, :], in_=ss[:])
            nc.sync.dma_start(out=out_d[130:131, :], in_=mus[:])
    if not nc.is_finalized():
        nc.finalize()
    return nc


def _host_inputs(angle_matrix, noise, nchunk):
    """Per-core input maps. Core c owns output rows [12c, 12c+12)."""
    pconst, rconst, sgconst = _host_constants()
    noise2d = np.ascontiguousarray(np.asarray(noise, dtype=np.float32)[0, 0])
    ang = np.asarray(angle_matrix, dtype=np.float32).reshape(-1).astype(np.float64)
    c = np.cos(ang); s = np.sin(ang)
    alpha = (c * c / A_**2 + s * s / B_**2).astype(np.float32)
    beta = (s * s / A_**2 + c * c / B_**2).astype(np.float32)
    gamma = (2 * c * s * (1 / A_**2 - 1 / B_**2)).astype(np.float32)
    x = np.linspace(-D, D, N, dtype=np.float32)
    x2 = x * x
    nwin = nchunk * CB
    qbasis = np.stack([x2, np.ones(N, np.float32), x]).astype(np.float32)  # [3, 65]

    in_maps = []
    for core in range(NCORE):
        w0 = core * WPC
        al = alpha[w0:w0 + nwin]
        be = beta[w0:w0 + nwin]
        ga = gamma[w0:w0 + nwin]
        # q[r, (w,c)] = x2[r]*rhsq[0] + 1*rhsq[1] + x[r]*rhsq[2]
        rhsq = np.empty((3, nwin, N), np.float32)
        rhsq[0] = al[:, None]
        rhsq[1] = be[:, None] * x2[None, :]
        rhsq[2] = ga[:, None] * x[None, :]
        r_base = core * RPC
        strips = np.concatenate(
            [noise2d[r_base + i: r_base + i + N, :] for i in range(12)], axis=1
        ).astype(_bf16)
        in_maps.append({
            "strips": strips,
            "rhsq": rhsq.reshape(3, -1),
            "qbasis": qbasis,
            "pconst": pconst,
            "rconst": rconst,
            "sgconst": sgconst,
        })
    return in_maps


def _finalize(core_outs):
    """core_outs: list of [131, nwin] arrays -> [96, 96] output."""
    blocks = []
    for arr in core_outs:
        vc = arr[0:N].sum(axis=0)
        s_ = arr[N:130].sum(axis=0)
        mu = arr[130]
        vcn = (vc - mu) / N2
        var = (s_ - mu * mu) / (N2 * (N2 - 1.0))
        outrow = vcn / (np.sqrt(np.maximum(var, 0.0)) + 1e-6)
        blocks.append(outrow.reshape(-1, 96))
    return np.concatenate(blocks, axis=0).astype(np.float32)


_PROG = {}


def _get_program(nchunk):
    if nchunk not in _PROG:
        _PROG[nchunk] = _build_program(nchunk)
    return _PROG[nchunk]


def kernel(angle_matrix, noise):
    from concourse.bass_utils import run_bass_kernel_spmd

    nchunk = WPC // CB
    nc = _get_program(nchunk)
    in_maps = _host_inputs(angle_matrix, noise, nchunk)
    res = run_bass_kernel_spmd(nc, in_maps, core_ids=list(range(NCORE)))
    core_outs = [res.results[i]["out"] for i in range(NCORE)]
    return _finalize(core_outs)


# revision 18
# speedup vs baseline: 66.6512x; 4.5775x over previous
"""Trainium2 Bass kernel for nn_BatchFFTMA: 9216 independent 65x65 FFT-MA sims.

Math (validated in proto.py against the jax reference):
  For each window w (patch p = noise[r0:r0+65, c0:c0+65], angle theta):
    Wf' = Cpt^T p Cpt   with Cpt = F*diag((-1)^k)  (patch DFT; center-pixel
          phase e^{2pi i 32k/65} and the principal-sqrt half-shift phase
          combine to exactly (-1)^k)
    E   = Re(Cq^T R Cq) with Cq = F*diag(e^{-2pi i 33 k/65})  (DFT of the
          ifftshift-aligned R -> real, even spectrum; the reference's
          fftshift-vs-ifftshift off-by-one is the source of the half-shift)
    R   = exp(-sqrt(q)), q = alpha*x_r^2 + beta*x_c^2 + gamma*x_r*x_c
    gp  = sqrt(relu(E+1e-8)), gn = sqrt(relu(-(E+1e-8)))
    g+  = gp*SGP, g- = gn*SGN   (SGP/SGN: +-1 fields from sqrt branch cuts)
    a = Wf'_r*g+, b = Wf'_i*g-; Xr = a - b
    VC = sum(Xr); X00 = Xr[0,0]; S = sum((Wf'_r^2+Wf'_i^2)*|E+1e-8|)
    out_w = ((VC-X00)/N^2) / (sqrt((S-X00^2)/(N^2(N^2-1))) + 1e-6)
  (v = ifft2 never materialized: center pixel via phase fold, mean via X[0,0],
   std via Parseval.)

Sharding: window/batch axis across 8 cores (1152 windows each, 12 output rows).
Precision: patch DFT + spectral chain bf16 (validated), R field + R DFT fp32.
"""
import os
import numpy as np
import ml_dtypes

H, W, D = 96, 96, 32
N = 65
N2 = N * N
A_, B_ = 15.0, 3.0
NCORE = 8
WPC = H * W // NCORE      # 1152 windows per core
RPC = H // NCORE          # 12 output rows per core
CB = 18                   # windows per vector chunk
GRP = 3                   # windows per matmul/PSUM group
NGRP = CB // GRP

_bf16 = ml_dtypes.bfloat16


def _host_constants():
    k = np.arange(N)
    F = np.exp(-2j * np.pi * np.outer(k, k) / N)
    Cpt = F * ((-1.0) ** k)[None, :]
    Cq = F * np.exp(-2j * np.pi * k * 33 / N)[None, :]
    Cr = Cpt.real.astype(np.float32)
    Ci = Cpt.imag.astype(np.float32)
    Qr = Cq.real.astype(np.float32)
    Qi = Cq.imag.astype(np.float32)
    pconst = np.concatenate([Cr, Ci, -Ci, Cr], axis=1).astype(_bf16)   # [65, 260]
    rconst = np.concatenate([Qr, Qi, -Qi], axis=1).astype(np.float32)  # [65, 195]

    k1, k2 = np.meshgrid(k, k, indexing="ij")
    ksum = k1 + k2
    m = ksum % N
    extra = (-1.0) ** (ksum // N)
    SGP = extra * np.where(m <= 32, 1.0, -1.0)
    SGN = extra * np.where(m == 0, 1.0, -1.0)
    sgp_t = np.tile(SGP, (1, CB)).astype(_bf16)   # [65, 65*CB]
    sgn_t = np.tile(SGN, (1, CB)).astype(_bf16)
    sgconst = np.concatenate([sgp_t, sgn_t], axis=1)  # [65, 2*65*CB]
    return pconst, rconst, sgconst


def _build_program(nchunk):
    import concourse.bacc as bacc
    import concourse.mybir as mybir
    from concourse.tile import TileContext

    f32 = mybir.dt.float32
    bf16 = mybir.dt.bfloat16
    AF = mybir.ActivationFunctionType
    ALU = mybir.AluOpType
    AX = mybir.AxisListType

    nwin = nchunk * CB
    nc = bacc.Bacc()
    strips_in = nc.declare_dram_parameter("strips", [N, 12 * 160], bf16, isOutput=False)
    rhsq_in = nc.declare_dram_parameter("rhsq", [3, nwin * N], f32, isOutput=False)
    qbasis_in = nc.declare_dram_parameter("qbasis", [3, N], f32, isOutput=False)
    pconst_in = nc.declare_dram_parameter("pconst", [N, 260], bf16, isOutput=False)
    rconst_in = nc.declare_dram_parameter("rconst", [N, 195], f32, isOutput=False)
    sg_in = nc.declare_dram_parameter("sgconst", [N, 2 * N * CB], bf16, isOutput=False)
    out_d = nc.declare_dram_parameter("out", [131, nwin], f32, isOutput=True)

    with TileContext(nc) as tc:
        with (
            tc.tile_pool(name="const", bufs=1) as cpool,
            tc.tile_pool(name="qsb", bufs=8) as qpool,
            tc.tile_pool(name="rsb", bufs=2) as rpool,
            tc.tile_pool(name="o1p", bufs=2) as o1ppool,
            tc.tile_pool(name="o1r", bufs=2) as o1rpool,
            tc.tile_pool(name="wfsb", bufs=2) as wfpool,
            tc.tile_pool(name="esb", bufs=2) as epool,
            tc.tile_pool(name="spec", bufs=2) as spool,
            tc.tile_pool(name="psq", bufs=2, space="PSUM") as pq_,
            tc.tile_pool(name="ps1p", bufs=2, space="PSUM") as pp1,
            tc.tile_pool(name="ps1r", bufs=2, space="PSUM") as pr1,
            tc.tile_pool(name="pswf", bufs=1, space="PSUM") as pwf,
            tc.tile_pool(name="pse", bufs=1, space="PSUM") as pe_,
        ):
            pc = cpool.tile([N, 260], bf16)
            nc.sync.dma_start(out=pc[:], in_=pconst_in[:])
            rc = cpool.tile([N, 195], f32)
            nc.sync.dma_start(out=rc[:], in_=rconst_in[:])
            sg = cpool.tile([N, 2 * N * CB], bf16)
            nc.sync.dma_start(out=sg[:], in_=sg_in[:])

            strips = cpool.tile([N, 12 * 160], bf16)
            nc.sync.dma_start(out=strips[:], in_=strips_in[:])
            qbasis = cpool.tile([3, N], f32)
            nc.sync.dma_start(out=qbasis[:], in_=qbasis_in[:])

            # clock warm-up: make ACT/DVE observe every setup DMA queue via
            # tiny reads, so later real instructions emit few sync waits
            # (walrus caps waits per instruction; vector clocks here are not
            # transitively collapsed).
            warm_s = cpool.tile([1, 4], f32, tag="warm_s")
            warm_v = cpool.tile([1, 4], f32, tag="warm_v")
            for idx, src in enumerate((strips, pc, rc, sg)):
                nc.scalar.copy(warm_s[0:1, idx:idx + 1], src[0:1, 0:1])
                nc.vector.tensor_copy(warm_v[0:1, idx:idx + 1], src[0:1, 0:1])

            vcs = cpool.tile([N, nwin], f32)
            ss = cpool.tile([N, nwin], f32)
            mus = cpool.tile([1, nwin], f32)

            for ch in range(nchunk):
                cw0 = ch * CB
                rq_sb = qpool.tile([3, CB * N], f32)
                nc.gpsimd.dma_start(
                    out=rq_sb[:], in_=rhsq_in[:, cw0 * N:(cw0 + CB) * N]
                )
                s_sb = qpool.tile([N, CB * N], f32, tag="s_sb")
                for h in range(3):
                    qps = pq_.tile([N, 390], f32)
                    nc.tensor.matmul(
                        qps[:], qbasis[:], rq_sb[:, h * 390:(h + 1) * 390],
                        start=True, stop=True,
                    )
                    # sqrt straight out of PSUM into the chunk-wide s tile
                    nc.scalar.activation(
                        s_sb[:, h * 390:(h + 1) * 390], qps[:], AF.Sqrt
                    )
                r_sb = rpool.tile([N, CB * N], f32)
                nc.scalar.activation(r_sb[:], s_sb[:], AF.Exp, scale=-1.0)

                wf_sb = wfpool.tile([N, CB * 130], bf16)
                e_sb = epool.tile([N, CB * N], bf16)

                for g in range(NGRP):
                    ps1p = pp1.tile([N, 390], f32)
                    ps1r = pr1.tile([N, 390], f32)
                    wfp = pwf.tile([N, 390], f32)
                    ep = pe_.tile([N, 195], f32)
                    for j in range(GRP):
                        w = cw0 + g * GRP + j
                        r0, c0 = divmod(w, 96)
                        lhs = strips[:, r0 * 160 + c0: r0 * 160 + c0 + N]
                        nc.tensor.matmul(
                            ps1p[:, j * 130:(j + 1) * 130], lhs, pc[:, 0:130],
                            start=True, stop=True,
                        )
                    o1p = o1ppool.tile([N, 390], bf16)
                    nc.scalar.copy(o1p[:], ps1p[:])
                    for j in range(GRP):
                        nc.tensor.matmul(
                            wfp[:, j * 130:(j + 1) * 130],
                            o1p[:, j * 130:j * 130 + N], pc[:, 0:130],
                            start=True, stop=False,
                        )
                        nc.tensor.matmul(
                            wfp[:, j * 130:(j + 1) * 130],
                            o1p[:, j * 130 + N:(j + 1) * 130], pc[:, 130:260],
                            start=False, stop=True,
                        )
                    nc.scalar.copy(wf_sb[:, g * 390:(g + 1) * 390], wfp[:])

                    for j in range(GRP):
                        wl = (g * GRP + j) * N
                        nc.tensor.matmul(
                            ps1r[:, j * 130:(j + 1) * 130],
                            r_sb[:, wl:wl + N], rc[:, 0:130],
                            start=True, stop=True,
                        )
                    o1r = o1rpool.tile([N, 390], f32)
                    nc.vector.tensor_copy(o1r[:], ps1r[:])
                    for j in range(GRP):
                        nc.tensor.matmul(
                            ep[:, j * N:(j + 1) * N],
                            o1r[:, j * 130:j * 130 + N], rc[:, 0:N],
                            start=True, stop=False,
                        )
                        nc.tensor.matmul(
                            ep[:, j * N:(j + 1) * N],
                            o1r[:, j * 130 + N:(j + 1) * 130], rc[:, 130:195],
                            start=False, stop=True,
                        )
                    nc.vector.tensor_copy(e_sb[:, g * 195:(g + 1) * 195], ep[:])

                # spectral chain, batched over the CB windows of this chunk
                FD = CB * N
                rp = spool.tile([N, FD], bf16, tag="rp")
                nc.vector.tensor_scalar(rp[:], e_sb[:], 1e-8, 0.0, op0=ALU.add, op1=ALU.max)
                mn = spool.tile([N, FD], bf16, tag="mn")
                nc.vector.tensor_scalar(mn[:], e_sb[:], 1e-8, 0.0, op0=ALU.add, op1=ALU.min)
                gp = spool.tile([N, FD], bf16, tag="gp")
                nc.scalar.activation(gp[:], rp[:], AF.Sqrt)
                gn = spool.tile([N, FD], bf16, tag="gn")
                nc.scalar.activation(gn[:], mn[:], AF.Sqrt, scale=-1.0)
                gps = spool.tile([N, FD], bf16, tag="gps")
                nc.vector.tensor_mul(gps[:], gp[:], sg[:, 0:FD])
                gns = spool.tile([N, FD], bf16, tag="gns")
                nc.vector.tensor_mul(gns[:], gn[:], sg[:, FD:2 * FD])

                wf3 = wf_sb[:].rearrange("p (w t c) -> p w t c", t=2, c=N)
                wr = wf3[:, :, 0, :]   # [65, CB, 65]
                wi = wf3[:, :, 1, :]
                gps3 = gps[:].rearrange("p (w c) -> p w c", c=N)
                gns3 = gns[:].rearrange("p (w c) -> p w c", c=N)
                a_t = spool.tile([N, FD], bf16, tag="a_t")
                a3 = a_t[:].rearrange("p (w c) -> p w c", c=N)
                nc.vector.tensor_mul(a3, wr, gps3)
                b_t = spool.tile([N, FD], bf16, tag="b_t")
                b3 = b_t[:].rearrange("p (w c) -> p w c", c=N)
                nc.vector.tensor_mul(b3, wi, gns3)
                xr = spool.tile([N, FD], bf16, tag="xr")
                nc.vector.tensor_sub(xr[:], a_t[:], b_t[:])

                wr2 = spool.tile([N, FD], bf16, tag="wr2")
                wr23 = wr2[:].rearrange("p (w c) -> p w c", c=N)
                nc.vector.tensor_mul(wr23, wr, wr)
                wi2 = spool.tile([N, FD], bf16, tag="wi2")
                wi23 = wi2[:].rearrange("p (w c) -> p w c", c=N)
                nc.vector.tensor_mul(wi23, wi, wi)
                w2s = spool.tile([N, FD], bf16, tag="w2s")
                nc.vector.tensor_add(w2s[:], wr2[:], wi2[:])
                az = spool.tile([N, FD], bf16, tag="az")
                nc.vector.tensor_sub(az[:], rp[:], mn[:])   # |E+eps| = relu(z) - min(z,0)
                st = spool.tile([N, FD], bf16, tag="st")
                nc.vector.tensor_mul(st[:], w2s[:], az[:])

                xr3 = xr[:].rearrange("p (w c) -> p w c", c=N)
                st3 = st[:].rearrange("p (w c) -> p w c", c=N)
                nc.vector.tensor_reduce(
                    vcs[:, cw0:cw0 + CB], xr3, axis=AX.X, op=ALU.add
                )
                nc.vector.tensor_reduce(
                    ss[:, cw0:cw0 + CB], st3, axis=AX.X, op=ALU.add
                )
                nc.vector.tensor_copy(mus[0:1, cw0:cw0 + CB], xr3[0:1, :, 0])

            nc.sync.dma_start(out=out_d[0:N, :], in_=vcs[:])
            nc.sync.dma_start(out=out_d[N:130, :], in_=ss[:])
            nc.sync.dma_start(out=out_d[130:131, :], in_=mus[:])
    if not nc.is_finalized():
        nc.finalize()
    return nc


def _host_inputs(angle_matrix, noise, nchunk):
    """Per-core input maps. Core c owns output rows [12c, 12c+12)."""
    pconst, rconst, sgconst = _host_constants()
    noise2d = np.ascontiguousarray(np.asarray(noise, dtype=np.float32)[0, 0])
    ang = np.asarray(angle_matrix, dtype=np.float32).reshape(-1).astype(np.float64)
    c = np.cos(ang); s = np.sin(ang)
    alpha = (c * c / A_**2 + s * s / B_**2).astype(np.float32)
    beta = (s * s / A_**2 + c * c / B_**2).astype(np.float32)
    gamma = (2 * c * s * (1 / A_**2 - 1 / B_**2)).astype(np.float32)
    x = np.linspace(-D, D, N, dtype=np.float32)
    x2 = x * x
    nwin = nchunk * CB
    qbasis = np.stack([x2, np.ones(N, np.float32), x]).astype(np.float32)  # [3, 65]

    in_maps = []
    for core in range(NCORE):
        w0 = core * WPC
        al = alpha[w0:w0 + nwin]
        be = beta[w0:w0 + nwin]
        ga = gamma[w0:w0 + nwin]
        # q[r, (w,c)] = x2[r]*rhsq[0] + 1*rhsq[1] + x[r]*rhsq[2]
        rhsq = np.empty((3, nwin, N), np.float32)
        rhsq[0] = al[:, None]
        rhsq[1] = be[:, None] * x2[None, :]
        rhsq[2] = ga[:, None] * x[None, :]
        r_base = core * RPC
        strips = np.concatenate(
            [noise2d[r_base + i: r_base + i + N, :] for i in range(12)], axis=1
        ).astype(_bf16)
        in_maps.append({
            "strips": strips,
            "rhsq": rhsq.reshape(3, -1),
            "qbasis": qbasis,
            "pconst": pconst,
            "rconst": rconst,
            "sgconst": sgconst,
        })
    return in_maps


def _finalize(core_outs):
    """core_outs: list of [131, nwin] arrays -> [96, 96] output."""
    blocks = []
    for arr in core_outs:
        vc = arr[0:N].sum(axis=0)
        s_ = arr[N:130].sum(axis=0)
        mu = arr[130]
        vcn = (vc - mu) / N2
        var = (s_ - mu * mu) / (N2 * (N2 - 1.0))
        outrow = vcn / (np.sqrt(np.maximum(var, 0.0)) + 1e-6)
        blocks.append(outrow.reshape(-1, 96))
    return np.concatenate(blocks, axis=0).astype(np.float32)


_PROG = {}
_RUNNER = {}


def _get_program(nchunk):
    if nchunk not in _PROG:
        _PROG[nchunk] = _build_program(nchunk)
    return _PROG[nchunk]


def _get_runner(nchunk):
    """Build (once) a jitted shard_map executable over the 8 cores.

    Mirrors concourse.bass2jax.run_bass_via_pjrt, but caches the traced/
    compiled callable so repeat kernel() calls skip retracing.
    """
    if nchunk in _RUNNER:
        return _RUNNER[nchunk]
    import jax
    import concourse.mybir as mybir
    from concourse import bass2jax
    from jax.experimental.shard_map import shard_map
    from jax.sharding import Mesh, PartitionSpec

    nc = _get_program(nchunk)
    bass2jax.install_neuronx_cc_hook()
    assert nc.dbg_addr is None
    partition_name = (
        nc.partition_id_tensor.name if nc.partition_id_tensor else None
    )
    in_names, out_names, out_avals, zero_outs = [], [], [], []
    for alloc in nc.m.functions[0].allocations:
        if not isinstance(alloc, mybir.MemoryLocationSet):
            continue
        name = alloc.memorylocations[0].name
        if alloc.kind == "ExternalInput":
            if name != partition_name:
                in_names.append(name)
        elif alloc.kind == "ExternalOutput":
            out_names.append(name)
            shape = tuple(alloc.tensor_shape)
            dtype = mybir.dt.np(alloc.dtype)
            out_avals.append(jax.core.ShapedArray(shape, dtype))
            zero_outs.append(np.zeros(shape, dtype))
    n_params = len(in_names)
    n_outs = len(out_avals)
    in_names_all = in_names + out_names
    if partition_name is not None:
        in_names_all.append(partition_name)
    donate = tuple(range(n_params, n_params + n_outs))

    def _body(*args):
        operands = list(args)
        if partition_name is not None:
            operands.append(bass2jax.partition_id_tensor())
        outs = bass2jax._bass_exec_p.bind(
            *operands,
            out_avals=tuple(out_avals),
            in_names=tuple(in_names_all),
            out_names=tuple(out_names),
            lowering_input_output_aliases=(),
            sim_require_finite=True,
            sim_require_nnan=True,
            nc=nc,
        )
        return tuple(outs)

    devices = jax.devices()[:NCORE]
    mesh = Mesh(np.asarray(devices), ("core",))
    sharded = jax.jit(
        shard_map(
            _body,
            mesh=mesh,
            in_specs=(PartitionSpec("core"),) * (n_params + n_outs),
            out_specs=(PartitionSpec("core"),) * n_outs,
            check_rep=False,
        ),
        donate_argnums=donate,
        keep_unused=True,
    )
    zero_concats = [
        np.zeros((NCORE * z.shape[0], *z.shape[1:]), z.dtype) for z in zero_outs
    ]
    info = (sharded, in_names, out_names, out_avals, zero_concats)
    _RUNNER[nchunk] = info
    return info


def _run(in_maps, nchunk):
    sharded, in_names, out_names, out_avals, zero_concats = _get_runner(nchunk)
    concat_in = [
        np.concatenate([in_maps[c][name] for c in range(NCORE)], axis=0)
        for name in in_names
    ]
    out_arrs = sharded(*concat_in, *[z.copy() for z in zero_concats])
    outs = {
        name: np.asarray(out_arrs[i]).reshape(NCORE, *out_avals[i].shape)
        for i, name in enumerate(out_names)
    }
    return [outs["out"][c] for c in range(NCORE)]


def kernel(angle_matrix, noise):
    nchunk = WPC // CB
    in_maps = _host_inputs(angle_matrix, noise, nchunk)
    core_outs = _run(in_maps, nchunk)
    return _finalize(core_outs)
